# revision 38
# baseline (speedup 1.0000x reference)
"""Multi-head attention kernel for TRN2, 8 NeuronCores, head-parallel.

Full problem: Q,K,V [B=4, H=8, S=4096, D=64] fp32; out = softmax(QK^T/8) V.
Sharding: 32 (b,h) slices -> 4 per core; no cross-core communication.

Per-core algorithm (heads processed in packed pairs A/B):
  - Prologue per pair, quartered, all off the compute engines' critical path.
    Pair 0 (latency-critical lead-in) uses HWDGE fp32 loads + DVE bf16 casts;
    pair 1 (220us of slack) uses gpsimd SWDGE DMAs that cast fp32->bf16 in
    flight with 2KB-contiguous-run descriptors.  Either way the bf16 rows
    bounce through a DRAM staging tensor that interleaves the two heads
    ([s, 2, 64]) so ONE 16-bit xbar-transpose DMA per quarter writes
    qt/kt [128, S] directly (head A on partitions 0:64, B on 64:128).
    (cast)->(store) ordering is Tile-tracked via the SBUF stage tile;
    (store)->(transpose) by same-queue FIFO.  V is cast+bounced the same way,
    with the final chunk-layout load ([128k, c, 65] with a ones column for
    the softmax denominator) on the fast HWDGE sync queue.
  - Main loop, one k-chunk (128) per step, software-pipelined with PV
    trailing QK by LAG=2 chunks (the in-order PE queue must never park on a
    PV whose exp isn't finished):
      scoresT[k, 2, q] <- one 2-bank psum supertile per chunk holding BOTH
      heads ([:,0,:] via kt rows 0:64, [:,1,:] via rows 64:128); the two QK
      matmuls are row-tile concurrent (disjoint row groups, shared stream).
      exp: ONE FD=1024 op per chunk covering both heads, alternating engines
      by chunk parity to halve the per-op overhead (352cyc ScalarE/120cyc
      DVE amortized over 1024 lanes):
        * kc even: ScalarE ACTIVATE exact exp (scale=1/8 folded), ~1.11us
        * kc odd:  VectorE tensor_scalar Schraudolph exp (i16 = round(A*s+B)
          bitcast bf16 ~= exp(s/8), ~2% element error, zero-mean so softmax
          renormalization cancels the bias), ~1.22us
      PV: lhsT = [V_chunk | ones] (65 cols); 2 matmuls (heads A/B) accumulate
      [65, 512] psum over 32 chunks.
  - Epilogue per (qb, head), split so the pso bank frees ASAP and the PE
    queue never parks: psum -> sbuf bf16 copies (A on ScalarE, B on VectorE)
    issued with the last PV; transpose-back (4 bf16 matmuls vs identity into
    scratch in the just-freed pso bank), reciprocal of the denominator row,
    broadcast multiply, and the store are deferred until just before the
    next q-block's accumulators are allocated (pso pool rotation:
    out_t(qb) -> ps4(qb) -> out_t(qb+1), hence the lazy out_t allocation).

PSUM budget (8 banks): score supertiles [128,1024] x3 = 6 banks, PV-out A/B
= 2 banks; epilogue transpose scratch reuses the freed pso banks.

Steady state ~878ns/chunk (512 chunks/core): PE streams 1536cyc = 640ns
(QK pair 512 + PV 2x512) + ~230ns exposed LDWEIGHTS (every matmul uses
conflicting row groups, so loads can't pull ahead); ScalarE ~620ns and
VectorE ~680ns per chunk ride under that.  Measured ~546us/core end-to-end.
"""

import numpy as np

from concourse import bacc, mybir, tile
from concourse.bass_utils import run_bass_kernel_spmd
from concourse.masks import make_identity

P = 128          # partitions
S = 4096         # sequence length
D = 64           # head dim
NH = 4           # heads per core
NC = S // P      # 32 k-chunks of 128
QB = 512         # q block (psum bank free size in fp32)
NQ = S // QB     # 8 q blocks
NQTR = 4         # DMA quarters
CPQ = NC // NQTR # chunks per quarter (8)
SQ = S // NQTR   # seq elems per quarter (1024)
FP32 = mybir.dt.float32
BF16 = mybir.dt.bfloat16
I16 = mybir.dt.int16
DP = 80           # padded PV output rows (64 d + 1 denom + 15 pad, 16-aligned)

N_CORES = 8
SCALE = 1.0 / np.sqrt(np.float32(D))  # 0.125

# Schraudolph exp-as-bf16-bits constants (see module docstring).
# i16 = round(EXP_A * s + EXP_B); bits -> bf16 ~= exp(s * SCALE).
EXP_A = float(128 * np.log2(np.e) * SCALE)
EXP_B = 16248.7807254998


def build():
    nc = bacc.Bacc("TRN2", target_bir_lowering=False)
    q_d = nc.dram_tensor("Q", (NH, S, D), FP32, kind="ExternalInput")
    k_d = nc.dram_tensor("K", (NH, S, D), FP32, kind="ExternalInput")
    v_d = nc.dram_tensor("V", (NH, S, D), FP32, kind="ExternalInput")
    o_d = nc.dram_tensor("out", (NH, S, D), FP32, kind="ExternalOutput")
    # DRAM bounce buffers for the bf16 transpose: rows interleave the two
    # heads of a pair ([s, 2, D] -> transpose input [s, 128]).
    qstg_d = nc.dram_tensor("qstg", (NH // 2, S, 2, D), BF16, kind="Internal")
    kstg_d = nc.dram_tensor("kstg", (NH // 2, S, 2, D), BF16, kind="Internal")
    vstg_d = nc.dram_tensor("vstg", (NH, S, D), BF16, kind="Internal")

    with tile.TileContext(nc) as tc:
        with (
            tc.tile_pool(name="const", bufs=1) as const_pool,
            tc.tile_pool(name="stg", bufs=3) as stg_pool,
            tc.tile_pool(name="stgf", bufs=3) as stgf_pool,
            tc.tile_pool(name="qt", bufs=2) as qt_pool,
            tc.tile_pool(name="kt", bufs=2) as kt_pool,
            tc.tile_pool(name="vsb", bufs=2) as vsb_pool,
            tc.tile_pool(name="pt", bufs=3) as pt_pool,
            tc.tile_pool(name="osb", bufs=4) as osb_pool,
            tc.tile_pool(name="fin", bufs=8) as fin_pool,
            tc.tile_pool(name="fin4", bufs=4) as fin4_pool,
            tc.tile_pool(name="recip", bufs=4) as recip_pool,
            tc.tile_pool(name="sc", bufs=3, space="PSUM") as sc_pool,
            tc.tile_pool(name="pso_a", bufs=1, space="PSUM") as pso_a_pool,
            tc.tile_pool(name="pso_b", bufs=1, space="PSUM") as pso_b_pool,
        ):
            ident = None

            def load_pair(pair):
                """Issue all loads for a pair (quartered), all on DMA engines.

                Per (tensor, head, quarter): gpsimd cast-DMA HBM fp32 ->
                SBUF bf16 rows; sync DMA SBUF -> DRAM bf16 staging; sync
                16-bit transpose DMA DRAM [SQ, D] -> qt/kt [64, SQ] slice.
                V: gpsimd cast-DMA straight into vsb.
                """
                ha, hb = 2 * pair, 2 * pair + 1
                qt = qt_pool.tile([P, S], BF16, name=f"qt_{pair}", tag="qt")
                kt = kt_pool.tile([P, S], BF16, name=f"kt_{pair}", tag="kt")
                vsb = vsb_pool.tile(
                    [P, 2, NC, DP], BF16, name=f"vsb_{pair}", tag="vsb"
                )
                nc.gpsimd.memset(vsb[:, :, :, D:DP], 0.0)
                nc.gpsimd.memset(vsb[:, :, :, D : D + 1], 1.0)

                def load_quarter(g, x_d, xstg_d, xt, hw_eng):
                    """3-hop bf16 transpose pipeline for one quarter of Q/K
                    (SWDGE cast path -- zero compute-engine cost)."""
                    lo = g * SQ
                    for h_i, h in enumerate((ha, hb)):
                        stg = stg_pool.tile([P, CPQ, D], BF16, tag="stg")
                        nc.gpsimd.dma_start(
                            out=stg,
                            in_=x_d[h][lo : lo + SQ, :].rearrange(
                                "(p c) d -> p c d", p=P
                            ),
                        )
                        hw_eng.dma_start(
                            out=xstg_d[pair, lo : lo + SQ, h_i, :].rearrange(
                                "(p c) d -> p c d", p=P
                            ),
                            in_=stg,
                        )
                    hw_eng.dma_start(
                        out=xt[:, lo : lo + SQ],
                        in_=xstg_d[pair, lo : lo + SQ].rearrange("s h d -> s (h d)"),
                        transpose=True,
                    )

                def load_quarter_hwdge(g, x_d, xstg_d, xt, hw_eng):
                    """Pair-0 lead-in variant: fp32 HWDGE load + DVE cast
                    (no SWDGE serialization; DVE is idle during lead-in)."""
                    lo = g * SQ
                    for h_i, h in enumerate((ha, hb)):
                        stgf = stgf_pool.tile([P, CPQ, D], FP32, tag="stgf")
                        hw_eng.dma_start(
                            out=stgf,
                            in_=x_d[h][lo : lo + SQ, :].rearrange(
                                "(p c) d -> p c d", p=P
                            ),
                        )
                        stg = stg_pool.tile([P, CPQ, D], BF16, tag="stg")
                        nc.vector.tensor_copy(stg, stgf)
                        hw_eng.dma_start(
                            out=xstg_d[pair, lo : lo + SQ, h_i, :].rearrange(
                                "(p c) d -> p c d", p=P
                            ),
                            in_=stg,
                        )
                    hw_eng.dma_start(
                        out=xt[:, lo : lo + SQ],
                        in_=xstg_d[pair, lo : lo + SQ].rearrange("s h d -> s (h d)"),
                        transpose=True,
                    )

                def load_v_quarter(g):
                    """SWDGE cast with big runs -> DRAM bf16 -> HWDGE
                    chunk-layout load."""
                    lo = g * SQ
                    clo = g * CPQ
                    for h_i, h in enumerate((ha, hb)):
                        stg = stg_pool.tile([P, CPQ, D], BF16, tag="stg")
                        nc.gpsimd.dma_start(
                            out=stg,
                            in_=v_d[h][lo : lo + SQ, :].rearrange(
                                "(p c) d -> p c d", p=P
                            ),
                        )
                        nc.sync.dma_start(
                            out=vstg_d[h][lo : lo + SQ, :].rearrange(
                                "(p c) d -> p c d", p=P
                            ),
                            in_=stg,
                        )
                        nc.sync.dma_start(
                            out=vsb[:, h_i, clo : clo + CPQ, 0:D],
                            in_=vstg_d[h][lo : lo + SQ, :].rearrange(
                                "(c p) d -> p c d", p=P
                            ),
                        )

                def load_v_quarter_hwdge(g):
                    """Pair-0 variant: fp32 chunk-layout HWDGE load (on the
                    vector queue, idle during lead-in) + DVE cast into vsb."""
                    clo = g * CPQ
                    for h_i, h in enumerate((ha, hb)):
                        stgf = stgf_pool.tile([P, CPQ, D], FP32, tag="stgf")
                        nc.scalar.dma_start(
                            out=stgf,
                            in_=v_d[h].rearrange("(c p) d -> p c d", p=P)[
                                :, clo : clo + CPQ, :
                            ],
                        )
                        nc.vector.tensor_copy(
                            vsb[:, h_i, clo : clo + CPQ, 0:D], stgf
                        )

                # K and V quarters are consumed progressively from kc=0, Q
                # quarter g only from qb=2g -- so load K/V first, one Q
                # quarter early, the rest at the end.  Pair 0's Q pipeline
                # rides the (idle until compute starts) scalar queue to
                # parallelize the critical lead-in; later pairs have slack
                # and keep everything on sync.
                if pair == 0:
                    lq, lv, q_eng = load_quarter_hwdge, load_v_quarter_hwdge, nc.scalar
                else:
                    lq, lv, q_eng = load_quarter, load_v_quarter, nc.sync
                for g in range(NQTR):
                    lq(g, k_d, kstg_d, kt, nc.sync)
                    lv(g)
                    if g == 0:
                        lq(0, q_d, qstg_d, qt, q_eng)
                for g in range(1, NQTR):
                    lq(g, q_d, qstg_d, qt, q_eng)
                return {"qt": qt, "kt": kt, "vsb": vsb, "heads": (ha, hb)}

            LAG = 2  # PV trails QK by LAG chunks (software pipelining: the
            # in-order PE queue must not park on a PV whose exp isn't done)

            def main_pair(st):
                qt, kt, vsb = st["qt"], st["kt"], st["vsb"]
                ha, hb = st["heads"]
                pend = []   # (pt, kc, out_ta, out_tb, qb) awaiting PV
                epil = []   # deferred epilogue tails

                def emit_epilogue_tail(osb, h, qd, pool, ptag):
                    """PE transpose into the pso bank (pool rotation:
                    out_t(qb) -> ps4(qb) -> out_t(qb+1), so out_t must be
                    lazily allocated after this), then normalize + store."""
                    ps4 = pool.tile(
                        [P, QB // P, D + 1], FP32, tag=ptag, name=f"ps4_{h}_{qd}"
                    )
                    for j in range(QB // P):
                        nc.tensor.matmul(
                            ps4[:, j, :],
                            lhsT=osb[:, j * P : (j + 1) * P],
                            rhs=ident[0:DP, 0 : D + 1],
                            start=True,
                            stop=True,
                        )
                    rec = recip_pool.tile([P, QB // P, 1], FP32, tag="rec")
                    nc.vector.reciprocal_approx_fast(rec, ps4[:, :, D : D + 1])
                    fin = fin_pool.tile([P, QB // P, D], FP32)
                    nc.vector.tensor_tensor(
                        fin,
                        ps4[:, :, 0:D],
                        rec.broadcast_to((P, QB // P, D)),
                        mybir.AluOpType.mult,
                    )
                    nc.sync.dma_start(
                        out=o_d[h, qd : qd + QB, :].rearrange(
                            "(j p) d -> p j d", p=P
                        ),
                        in_=fin,
                    )

                outs = {}

                def issue_pv(pt, kc, qb):
                    first = kc == 0
                    last = kc == NC - 1
                    if first:
                        # emit the previous qb's epilogue tails BEFORE
                        # allocating this qb's accumulators, so the pso pool
                        # rotation is out_t(qb-1) -> ps4(qb-1) -> out_t(qb)
                        while epil:
                            emit_epilogue_tail(*epil.pop(0))
                        outs[qb] = (
                            pso_a_pool.tile(
                                [DP, QB], FP32, tag="pso_a", name=f"ota_{ha}_{qb}"
                            ),
                            pso_b_pool.tile(
                                [DP, QB], FP32, tag="pso_b", name=f"otb_{ha}_{qb}"
                            ),
                        )
                    out_ta, out_tb = outs[qb]
                    nc.tensor.matmul(
                        out_ta,
                        lhsT=vsb[:, 0, kc, :],
                        rhs=pt[:, 0, :],
                        start=first,
                        stop=last,
                    )
                    nc.tensor.matmul(
                        out_tb,
                        lhsT=vsb[:, 1, kc, :],
                        rhs=pt[:, 1, :],
                        start=first,
                        stop=last,
                    )
                    if last:
                        # free the pso banks ASAP: copies now, rest deferred
                        qd = qb * QB
                        for h_i, (h, out_t, pool, ptag) in enumerate((
                            (ha, out_ta, pso_a_pool, "pso_a"),
                            (hb, out_tb, pso_b_pool, "pso_b"),
                        )):
                            osb = osb_pool.tile([DP, QB], BF16)
                            nc.scalar.copy(
                                osb[:, 0 : QB // 2], out_t[:, 0 : QB // 2]
                            )
                            nc.vector.tensor_copy(
                                osb[:, QB // 2 :], out_t[:, QB // 2 :]
                            )
                            epil.append((osb, h, qd, pool, ptag))

                for qb in range(NQ):
                    q0 = qb * QB
                    for kc in range(NC):
                        sc = sc_pool.tile([P, 2, QB], FP32, tag="sc")
                        nc.tensor.matmul(
                            sc[:, 0, :],
                            lhsT=kt[0:64, kc * P : (kc + 1) * P],
                            rhs=qt[0:64, q0 : q0 + QB],
                            start=True,
                            stop=True,
                        )
                        nc.tensor.matmul(
                            sc[:, 1, :],
                            lhsT=kt[64:128, kc * P : (kc + 1) * P],
                            rhs=qt[64:128, q0 : q0 + QB],
                            start=True,
                            stop=True,
                        )
                        if kc % 2 == 0:
                            pt = pt_pool.tile([P, 2, QB], BF16, tag="pt")
                            nc.scalar.activation(
                                pt,
                                sc,
                                mybir.ActivationFunctionType.Exp,
                                scale=SCALE,
                            )
                        else:
                            pt_i = pt_pool.tile([P, 2, QB], I16, tag="pt")
                            nc.vector.tensor_scalar(
                                out=pt_i,
                                in0=sc,
                                scalar1=EXP_A,
                                scalar2=EXP_B,
                                op0=mybir.AluOpType.mult,
                                op1=mybir.AluOpType.add,
                            )
                            pt = pt_i.bitcast(BF16)
                        pend.append((pt, kc, qb))
                        if len(pend) > LAG:
                            issue_pv(*pend.pop(0))
                for args in pend:
                    issue_pv(*args)
                while epil:
                    emit_epilogue_tail(*epil.pop(0))

            st0 = load_pair(0)
            # constants + exp table preload issued after the pair-0 load DMAs
            # so the lead-in queues aren't delayed by the preamble
            ident = const_pool.tile([P, P], BF16)
            make_identity(nc, ident)
            tl_src = const_pool.tile([P, 1], FP32)
            nc.vector.memset(tl_src, 0.0)
            tl_dst = const_pool.tile([P, 1], FP32)
            nc.scalar.activation(
                tl_dst, tl_src, mybir.ActivationFunctionType.Exp, scale=1.0
            )
            st1 = load_pair(1)
            main_pair(st0)
            main_pair(st1)

    nc.compile()
    return nc


_NC_CACHE = None


def _get_nc():
    global _NC_CACHE
    if _NC_CACHE is None:
        _NC_CACHE = build()
    return _NC_CACHE


def kernel(Q, K, V):
    Q = np.ascontiguousarray(np.asarray(Q, dtype=np.float32))
    K = np.ascontiguousarray(np.asarray(K, dtype=np.float32))
    V = np.ascontiguousarray(np.asarray(V, dtype=np.float32))
    B, H = Q.shape[0], Q.shape[1]
    qr = Q.reshape(B * H, S, D)
    kr = K.reshape(B * H, S, D)
    vr = V.reshape(B * H, S, D)
    in_maps = [
        {
            "Q": qr[i * NH : (i + 1) * NH],
            "K": kr[i * NH : (i + 1) * NH],
            "V": vr[i * NH : (i + 1) * NH],
        }
        for i in range(N_CORES)
    ]
    res = run_bass_kernel_spmd(_get_nc(), in_maps, core_ids=list(range(N_CORES)))
    out = np.concatenate([res.results[i]["out"] for i in range(N_CORES)], axis=0)
    return out.reshape(B, H, S, D)


# revision 39
# speedup vs baseline: 1.0309x; 1.0309x over previous
"""Multi-head attention kernel for TRN2, 8 NeuronCores, head-parallel.

Full problem: Q,K,V [B=4, H=8, S=4096, D=64] fp32; out = softmax(QK^T/8) V.
Sharding: 32 (b,h) slices -> 4 per core; no cross-core communication.

Per-core algorithm (heads processed in packed pairs A/B):
  - Prologue per pair, quartered, off the compute engines' critical path.
    Pair 0 (latency-critical lead-in) uses HWDGE fp32 loads + DVE bf16 casts;
    pair 1 (220us of slack) uses gpsimd SWDGE DMAs that cast fp32->bf16 in
    flight with 2KB-contiguous-run descriptors.  Either way the bf16 rows
    bounce through a DRAM staging tensor that interleaves the two heads
    ([s, 2, 64]) so ONE 16-bit xbar-transpose DMA per quarter writes
    qt/kt [128, S] directly (head A on partitions 0:64, B on 64:128).
    (cast)->(store) ordering is Tile-tracked via the SBUF stage tile;
    (store)->(transpose) by same-queue FIFO.  V is cast+bounced the same way,
    landing in chunk layout [128k, c, 65] with a ones column (softmax
    denominator accumulates free as row 64 of the PV output).
  - Main loop, one k-chunk (128) per step, software-pipelined with PV
    trailing QK by LAG=2 chunks (the in-order PE queue must never park on a
    PV whose exp isn't finished):
      scoresT[k, 2, q] <- one 2-bank psum supertile per chunk holding BOTH
      heads ([:,0,:] via kt rows 0:64, [:,1,:] via rows 64:128); the two QK
      matmuls are row-tile concurrent (disjoint row groups, shared stream).
      exp: ONE FD=1024 op per chunk covering both heads, alternating engines
      by chunk parity to halve the per-op overhead:
        * kc even: ScalarE ACTIVATE exact exp (scale=1/8 folded), ~1.11us
        * kc odd:  VectorE tensor_scalar Schraudolph exp (i16 = round(A*s+B)
          bitcast bf16 ~= exp(s/8), ~2% element error, zero-mean so softmax
          renormalization cancels the bias), ~1.22us
      PV: lhsT = [V_chunk | ones] (65 cols, padded to 80 rows); 2 matmuls
      (heads A/B) accumulate [80, 512] psum over 32 chunks.
  - Epilogue per (qb, head), split so the pso bank frees ASAP and the PE
    queue never parks: psum -> sbuf bf16 copies (A on ScalarE, B on VectorE)
    issued with the last PV; transpose-back (4 bf16 matmuls vs identity into
    scratch in the just-freed pso bank), reciprocal of the denominator row,
    broadcast multiply, and the store (sync queue) are deferred until just
    before the next q-block's accumulators are allocated (pso pool rotation:
    out_t(qb) -> ps4(qb) -> out_t(qb+1), hence the lazy out_t allocation).

PSUM budget (8 banks): score supertiles [128,1024] x3 = 6 banks, PV-out A/B
= 2 banks; epilogue transpose scratch reuses the freed pso banks.

Steady state ~878ns/chunk (512 chunks/core): PE streams 1536cyc = 640ns
(QK pair 512 + PV 2x512) + ~230ns exposed LDWEIGHTS (every matmul uses
conflicting row groups, so loads can't pull ahead); ScalarE ~620ns and
VectorE ~680ns per chunk ride under that.  Measured 546-562us/core.

Negative results (tried, reverted): TRN2 matmul cannot write bf16 psum
(fp32 only) so 2x-accel exp reads are impossible; SBUF-source xbar
transpose DMAs fault the device; DMA cannot touch PSUM; stride-0
(broadcast) DMA APs are rejected; DVE lanes are hardwired to partitions so
transposed-orientation normalization cannot consume a denominator row from
other partitions; fp8 DoubleRow PV exceeds the 2e-2 error budget (e4m3
probs ~2.5% output error); LAG=3 and epilogue copy-splitting regressed.
"""

import numpy as np

from concourse import bacc, mybir, tile
from concourse.bass_utils import run_bass_kernel_spmd
from concourse.masks import make_identity

P = 128          # partitions
S = 4096         # sequence length
D = 64           # head dim
NH = 4           # heads per core
NC = S // P      # 32 k-chunks of 128
QB = 512         # q block (psum bank free size in fp32)
NQ = S // QB     # 8 q blocks
NQTR = 4         # DMA quarters
CPQ = NC // NQTR # chunks per quarter (8)
SQ = S // NQTR   # seq elems per quarter (1024)
FP32 = mybir.dt.float32
BF16 = mybir.dt.bfloat16
I16 = mybir.dt.int16
DP = 80           # padded PV output rows (64 d + 1 denom + 15 pad, 16-aligned)

N_CORES = 8
SCALE = 1.0 / np.sqrt(np.float32(D))  # 0.125

# Schraudolph exp-as-bf16-bits constants (see module docstring).
# i16 = round(EXP_A * s + EXP_B); bits -> bf16 ~= exp(s * SCALE).
EXP_A = float(128 * np.log2(np.e) * SCALE)
EXP_B = 16248.7807254998


def build():
    nc = bacc.Bacc("TRN2", target_bir_lowering=False)
    q_d = nc.dram_tensor("Q", (NH, S, D), FP32, kind="ExternalInput")
    k_d = nc.dram_tensor("K", (NH, S, D), FP32, kind="ExternalInput")
    v_d = nc.dram_tensor("V", (NH, S, D), FP32, kind="ExternalInput")
    o_d = nc.dram_tensor("out", (NH, S, D), FP32, kind="ExternalOutput")
    # DRAM bounce buffers for the bf16 transpose: rows interleave the two
    # heads of a pair ([s, 2, D] -> transpose input [s, 128]).
    qstg_d = nc.dram_tensor("qstg", (NH // 2, S, 2, D), BF16, kind="Internal")
    kstg_d = nc.dram_tensor("kstg", (NH // 2, S, 2, D), BF16, kind="Internal")
    vstg_d = nc.dram_tensor("vstg", (NH, S, D), BF16, kind="Internal")

    with tile.TileContext(nc) as tc:
        with (
            tc.tile_pool(name="const", bufs=1) as const_pool,
            tc.tile_pool(name="stg", bufs=3) as stg_pool,
            tc.tile_pool(name="stgf", bufs=3) as stgf_pool,
            tc.tile_pool(name="qt", bufs=2) as qt_pool,
            tc.tile_pool(name="kt", bufs=2) as kt_pool,
            tc.tile_pool(name="vsb", bufs=2) as vsb_pool,
            tc.tile_pool(name="pt", bufs=3) as pt_pool,
            tc.tile_pool(name="osb", bufs=4) as osb_pool,
            tc.tile_pool(name="fin", bufs=8) as fin_pool,
            tc.tile_pool(name="fin4", bufs=4) as fin4_pool,
            tc.tile_pool(name="recip", bufs=4) as recip_pool,
            tc.tile_pool(name="sc", bufs=3, space="PSUM") as sc_pool,
            tc.tile_pool(name="pso_a", bufs=1, space="PSUM") as pso_a_pool,
            tc.tile_pool(name="pso_b", bufs=1, space="PSUM") as pso_b_pool,
        ):
            ident = const_pool.tile([P, P], BF16)
            make_identity(nc, ident)

            # preload the exp table-set (~2.7us) before any data arrives
            tl_src = const_pool.tile([P, 1], FP32)
            nc.vector.memset(tl_src, 0.0)
            tl_dst = const_pool.tile([P, 1], FP32)
            nc.scalar.activation(
                tl_dst, tl_src, mybir.ActivationFunctionType.Exp, scale=1.0
            )

            def load_pair(pair):
                """Issue all loads for a pair (quartered), all on DMA engines.

                Per (tensor, head, quarter): gpsimd cast-DMA HBM fp32 ->
                SBUF bf16 rows; sync DMA SBUF -> DRAM bf16 staging; sync
                16-bit transpose DMA DRAM [SQ, D] -> qt/kt [64, SQ] slice.
                V: gpsimd cast-DMA straight into vsb.
                """
                ha, hb = 2 * pair, 2 * pair + 1
                qt = qt_pool.tile([P, S], BF16, name=f"qt_{pair}", tag="qt")
                kt = kt_pool.tile([P, S], BF16, name=f"kt_{pair}", tag="kt")
                vsb = vsb_pool.tile(
                    [P, 2, NC, DP], BF16, name=f"vsb_{pair}", tag="vsb"
                )
                nc.gpsimd.memset(vsb[:, :, :, D:DP], 0.0)
                nc.gpsimd.memset(vsb[:, :, :, D : D + 1], 1.0)

                def load_quarter(g, x_d, xstg_d, xt, hw_eng):
                    """3-hop bf16 transpose pipeline for one quarter of Q/K
                    (SWDGE cast path -- zero compute-engine cost)."""
                    lo = g * SQ
                    for h_i, h in enumerate((ha, hb)):
                        stg = stg_pool.tile([P, CPQ, D], BF16, tag="stg")
                        nc.gpsimd.dma_start(
                            out=stg,
                            in_=x_d[h][lo : lo + SQ, :].rearrange(
                                "(p c) d -> p c d", p=P
                            ),
                        )
                        hw_eng.dma_start(
                            out=xstg_d[pair, lo : lo + SQ, h_i, :].rearrange(
                                "(p c) d -> p c d", p=P
                            ),
                            in_=stg,
                        )
                    hw_eng.dma_start(
                        out=xt[:, lo : lo + SQ],
                        in_=xstg_d[pair, lo : lo + SQ].rearrange("s h d -> s (h d)"),
                        transpose=True,
                    )

                def load_quarter_hwdge(g, x_d, xstg_d, xt, hw_eng):
                    """Pair-0 lead-in variant: fp32 HWDGE load + DVE cast
                    (no SWDGE serialization; DVE is idle during lead-in)."""
                    lo = g * SQ
                    for h_i, h in enumerate((ha, hb)):
                        stgf = stgf_pool.tile([P, CPQ, D], FP32, tag="stgf")
                        hw_eng.dma_start(
                            out=stgf,
                            in_=x_d[h][lo : lo + SQ, :].rearrange(
                                "(p c) d -> p c d", p=P
                            ),
                        )
                        stg = stg_pool.tile([P, CPQ, D], BF16, tag="stg")
                        nc.vector.tensor_copy(stg, stgf)
                        hw_eng.dma_start(
                            out=xstg_d[pair, lo : lo + SQ, h_i, :].rearrange(
                                "(p c) d -> p c d", p=P
                            ),
                            in_=stg,
                        )
                    hw_eng.dma_start(
                        out=xt[:, lo : lo + SQ],
                        in_=xstg_d[pair, lo : lo + SQ].rearrange("s h d -> s (h d)"),
                        transpose=True,
                    )

                def load_v_quarter(g):
                    """SWDGE cast with big runs -> DRAM bf16 -> HWDGE
                    chunk-layout load."""
                    lo = g * SQ
                    clo = g * CPQ
                    for h_i, h in enumerate((ha, hb)):
                        stg = stg_pool.tile([P, CPQ, D], BF16, tag="stg")
                        nc.gpsimd.dma_start(
                            out=stg,
                            in_=v_d[h][lo : lo + SQ, :].rearrange(
                                "(p c) d -> p c d", p=P
                            ),
                        )
                        nc.sync.dma_start(
                            out=vstg_d[h][lo : lo + SQ, :].rearrange(
                                "(p c) d -> p c d", p=P
                            ),
                            in_=stg,
                        )
                        nc.sync.dma_start(
                            out=vsb[:, h_i, clo : clo + CPQ, 0:D],
                            in_=vstg_d[h][lo : lo + SQ, :].rearrange(
                                "(c p) d -> p c d", p=P
                            ),
                        )

                def load_v_quarter_hwdge(g):
                    """Pair-0 variant: fp32 chunk-layout HWDGE load + DVE
                    cast into vsb."""
                    clo = g * CPQ
                    for h_i, h in enumerate((ha, hb)):
                        stgf = stgf_pool.tile([P, CPQ, D], FP32, tag="stgf")
                        nc.sync.dma_start(
                            out=stgf,
                            in_=v_d[h].rearrange("(c p) d -> p c d", p=P)[
                                :, clo : clo + CPQ, :
                            ],
                        )
                        nc.vector.tensor_copy(
                            vsb[:, h_i, clo : clo + CPQ, 0:D], stgf
                        )

                # K and V quarters are consumed progressively from kc=0, Q
                # quarter g only from qb=2g -- so load K/V first, one Q
                # quarter early, the rest at the end.  Pair 0's Q pipeline
                # rides the (idle until compute starts) scalar queue to
                # parallelize the critical lead-in; later pairs have slack
                # and keep everything on sync.
                if pair == 0:
                    lq, lv, q_eng = load_quarter_hwdge, load_v_quarter_hwdge, nc.scalar
                else:
                    lq, lv, q_eng = load_quarter, load_v_quarter, nc.sync
                for g in range(NQTR):
                    lq(g, k_d, kstg_d, kt, nc.sync)
                    lv(g)
                    if g == 0:
                        lq(0, q_d, qstg_d, qt, q_eng)
                for g in range(1, NQTR):
                    lq(g, q_d, qstg_d, qt, q_eng)
                return {"qt": qt, "kt": kt, "vsb": vsb, "heads": (ha, hb)}

            LAG = 2  # PV trails QK by LAG chunks (software pipelining: the
            # in-order PE queue must not park on a PV whose exp isn't done)

            def main_pair(st):
                qt, kt, vsb = st["qt"], st["kt"], st["vsb"]
                ha, hb = st["heads"]
                pend = []   # (pt, kc, out_ta, out_tb, qb) awaiting PV
                epil = []   # deferred epilogue tails

                def emit_epilogue_tail(osb, h, qd, pool, ptag):
                    """PE transpose into the pso bank (pool rotation:
                    out_t(qb) -> ps4(qb) -> out_t(qb+1), so out_t must be
                    lazily allocated after this), then normalize + store."""
                    ps4 = pool.tile(
                        [P, QB // P, D + 1], FP32, tag=ptag, name=f"ps4_{h}_{qd}"
                    )
                    for j in range(QB // P):
                        nc.tensor.matmul(
                            ps4[:, j, :],
                            lhsT=osb[:, j * P : (j + 1) * P],
                            rhs=ident[0:DP, 0 : D + 1],
                            start=True,
                            stop=True,
                        )
                    rec = recip_pool.tile([P, QB // P, 1], FP32, tag="rec")
                    nc.vector.reciprocal_approx_fast(rec, ps4[:, :, D : D + 1])
                    fin = fin_pool.tile([P, QB // P, D], FP32)
                    nc.vector.tensor_tensor(
                        fin,
                        ps4[:, :, 0:D],
                        rec.broadcast_to((P, QB // P, D)),
                        mybir.AluOpType.mult,
                    )
                    nc.sync.dma_start(
                        out=o_d[h, qd : qd + QB, :].rearrange(
                            "(j p) d -> p j d", p=P
                        ),
                        in_=fin,
                    )

                outs = {}

                def issue_pv(pt, kc, qb):
                    first = kc == 0
                    last = kc == NC - 1
                    if first:
                        # emit the previous qb's epilogue tails BEFORE
                        # allocating this qb's accumulators, so the pso pool
                        # rotation is out_t(qb-1) -> ps4(qb-1) -> out_t(qb)
                        while epil:
                            emit_epilogue_tail(*epil.pop(0))
                        outs[qb] = (
                            pso_a_pool.tile(
                                [DP, QB], FP32, tag="pso_a", name=f"ota_{ha}_{qb}"
                            ),
                            pso_b_pool.tile(
                                [DP, QB], FP32, tag="pso_b", name=f"otb_{ha}_{qb}"
                            ),
                        )
                    out_ta, out_tb = outs[qb]
                    nc.tensor.matmul(
                        out_ta,
                        lhsT=vsb[:, 0, kc, :],
                        rhs=pt[:, 0, :],
                        start=first,
                        stop=last,
                    )
                    nc.tensor.matmul(
                        out_tb,
                        lhsT=vsb[:, 1, kc, :],
                        rhs=pt[:, 1, :],
                        start=first,
                        stop=last,
                    )
                    if last:
                        # free the pso banks ASAP: copies now, rest deferred
                        qd = qb * QB
                        for h_i, (h, out_t, pool, ptag) in enumerate((
                            (ha, out_ta, pso_a_pool, "pso_a"),
                            (hb, out_tb, pso_b_pool, "pso_b"),
                        )):
                            osb = osb_pool.tile([DP, QB], BF16)
                            if h_i == 0:
                                nc.scalar.copy(osb, out_t)
                            else:
                                nc.vector.tensor_copy(osb, out_t)
                            epil.append((osb, h, qd, pool, ptag))

                for qb in range(NQ):
                    q0 = qb * QB
                    for kc in range(NC):
                        sc = sc_pool.tile([P, 2, QB], FP32, tag="sc")
                        nc.tensor.matmul(
                            sc[:, 0, :],
                            lhsT=kt[0:64, kc * P : (kc + 1) * P],
                            rhs=qt[0:64, q0 : q0 + QB],
                            start=True,
                            stop=True,
                        )
                        nc.tensor.matmul(
                            sc[:, 1, :],
                            lhsT=kt[64:128, kc * P : (kc + 1) * P],
                            rhs=qt[64:128, q0 : q0 + QB],
                            start=True,
                            stop=True,
                        )
                        if kc % 2 == 0:
                            pt = pt_pool.tile([P, 2, QB], BF16, tag="pt")
                            nc.scalar.activation(
                                pt,
                                sc,
                                mybir.ActivationFunctionType.Exp,
                                scale=SCALE,
                            )
                        else:
                            pt_i = pt_pool.tile([P, 2, QB], I16, tag="pt")
                            nc.vector.tensor_scalar(
                                out=pt_i,
                                in0=sc,
                                scalar1=EXP_A,
                                scalar2=EXP_B,
                                op0=mybir.AluOpType.mult,
                                op1=mybir.AluOpType.add,
                            )
                            pt = pt_i.bitcast(BF16)
                        pend.append((pt, kc, qb))
                        if len(pend) > LAG:
                            issue_pv(*pend.pop(0))
                for args in pend:
                    issue_pv(*args)
                while epil:
                    emit_epilogue_tail(*epil.pop(0))

            st0 = load_pair(0)
            st1 = load_pair(1)
            main_pair(st0)
            main_pair(st1)

    nc.compile()
    return nc


_NC_CACHE = None


def _get_nc():
    global _NC_CACHE
    if _NC_CACHE is None:
        _NC_CACHE = build()
    return _NC_CACHE


def kernel(Q, K, V):
    Q = np.ascontiguousarray(np.asarray(Q, dtype=np.float32))
    K = np.ascontiguousarray(np.asarray(K, dtype=np.float32))
    V = np.ascontiguousarray(np.asarray(V, dtype=np.float32))
    B, H = Q.shape[0], Q.shape[1]
    qr = Q.reshape(B * H, S, D)
    kr = K.reshape(B * H, S, D)
    vr = V.reshape(B * H, S, D)
    in_maps = [
        {
            "Q": qr[i * NH : (i + 1) * NH],
            "K": kr[i * NH : (i + 1) * NH],
            "V": vr[i * NH : (i + 1) * NH],
        }
        for i in range(N_CORES)
    ]
    res = run_bass_kernel_spmd(_get_nc(), in_maps, core_ids=list(range(N_CORES)))
    out = np.concatenate([res.results[i]["out"] for i in range(N_CORES)], axis=0)
    return out.reshape(B, H, S, D)
